# revision 15
# baseline (speedup 1.0000x reference)
"""DeepseekV3 MLA attention prefill on 8 Trainium2 NeuronCores.

Sharding: batch x heads. Cores 0-3 run batch 0, cores 4-7 batch 1; within
a batch group each core owns 4 of the 16 heads. The replicated front-end
(q_a / kv_a projections) is computed once per core for its single batch
(vs twice in a pure head-parallel split), cutting per-core matmul work by
~27%. The final wo partial outputs are summed on the host per group.

Inside each core: transposed dataflow (contraction dims on SBUF
partitions), bf16 matmuls with f32 PSUM accumulation, f32r (full-rate
fp32) for the final wo projection, causal flash-style attention without
max-subtraction (scores ~N(0,0.3), exp is safe in f32). The score loop is
software-pipelined: score matmuls for block i+1 are emitted ahead of the
exp-dependent po matmuls of block i so the PE never idles on the scalar
engine. Rope is folded into host-side weight transforms.
"""
import os
import sys
import types

import numpy as np

# --- environment bootstrap (idempotent) --------------------------------
for _p in ("/opt/trn_rl_repo",):
    if os.path.isdir(_p) and _p not in sys.path:
        sys.path.insert(0, _p)
_B16 = ("/nix/store/wxap7svlj45h0lfm31d1axjjnzyl6qsy-b16-bazel-unstable-cc-"
        "2026-05-04-9a3fa1f3-rt-2026-05-04-ade39e0a/lib/python3.13/site-packages")
if os.path.isdir(_B16) and _B16 not in sys.path:
    sys.path.insert(0, _B16)

if "antenv.axon_hooks" not in sys.modules:
    try:
        import antenv

        _mod = types.ModuleType("antenv.axon_hooks")
        _hook = [None]
        _mod.set_axon_ntff_profile_hook = lambda h: _hook.__setitem__(0, h)
        _mod.get_axon_ntff_profile_hook = lambda: _hook[0]
        sys.modules["antenv.axon_hooks"] = _mod
        antenv.axon_hooks = _mod
        try:
            from trn_agent_boot.trn_boot import _ntff_profile_via_ctypes

            _mod.set_axon_ntff_profile_hook(
                _ntff_profile_via_ctypes("/opt/axon/libaxon_pjrt.so"))
        except Exception:
            pass
    except Exception:
        pass

import ml_dtypes

import concourse.bass as bass
import concourse.mybir as mybir
import concourse.tile as tile
from concourse.bass_utils import run_bass_kernel_spmd
from concourse.masks import make_identity

f32 = mybir.dt.float32
f32r = mybir.dt.float32r
bf16 = mybir.dt.bfloat16
f8 = mybir.dt.float8e4
EXP = mybir.ActivationFunctionType.Exp
DR = mybir.MatmulPerfMode.DoubleRow
SQRT = mybir.ActivationFunctionType.Sqrt
BF16NP = ml_dtypes.bfloat16

B, S, HID = 2, 2048, 2048
NH, NCORES = 16, 8
GROUP = 4             # cores per batch
HPC = NH // GROUP     # heads per core (4)
Q_LORA, KV_LORA = 1536, 512
NOPE, ROPE_D, VH = 128, 64, 128
EPS = 1e-6
THETA = 10000.0
SCALE = (NOPE + ROPE_D) ** -0.5
Q4 = 4.0               # q-side pre-scale so fp8 q_abs avoids denormals

LAST_EXEC_NS = None
_BUILD_CACHE = {}


# ----------------------------------------------------------------------
# device program (SPMD; one Bass program, per-core weights via in_maps)
# ----------------------------------------------------------------------
def _build_program(s=S):
    nt = s // 512          # 512-token tiles
    ntc = s // 128         # 128-token chunks

    nc = bass.Bass()
    d_hid = nc.declare_dram_parameter("hidden", [s, HID], bf16, isOutput=False)
    d_wqaT = nc.declare_dram_parameter("wqaT", [8, 128, 2, Q_LORA], f8,
                                       isOutput=False)
    d_wkvaT = nc.declare_dram_parameter("wkvaT", [HID, 640], bf16, isOutput=False)
    d_wqbT = nc.declare_dram_parameter("wqbT", [6, 128, 2, 1024], f8,
                                       isOutput=False)
    d_qabs = nc.declare_dram_parameter("qabs", [HPC, 128, 512], bf16, isOutput=False)
    d_oabsT = nc.declare_dram_parameter("oabsT", [HPC, 512, 128], bf16, isOutput=False)
    d_woT = nc.declare_dram_parameter("woT", [HPC * VH, HID], f32, isOutput=False)
    d_cosT = nc.declare_dram_parameter("cosT", [128, s], bf16, isOutput=False)
    d_sinT = nc.declare_dram_parameter("sinT", [128, s], bf16, isOutput=False)
    d_mask = nc.declare_dram_parameter("maskT", [4, 128, 512], bf16, isOutput=False)
    d_out = nc.declare_dram_parameter("out", [HID, s], f32, isOutput=True)

    with tile.TileContext(nc) as tc:
        with tc.tile_pool(name="tables", bufs=1) as tp:
            ident = tp.tile([128, 128], bf16, tag="ident")
            make_identity(nc, ident[:])
            cosT = tp.tile([128, s], bf16, tag="cosT")
            sinT = tp.tile([128, s], bf16, tag="sinT")
            nc.gpsimd.dma_start(out=cosT[:], in_=d_cosT[:])
            nc.gpsimd.dma_start(out=sinT[:], in_=d_sinT[:])
            masks = []
            for k in range(4):
                m = tp.tile([128, 512], bf16, tag=f"mask{k}")
                nc.gpsimd.dma_start(out=m[:], in_=d_mask[k])
                masks.append(m)
            ones_bf = tp.tile([128, 1], bf16, tag="ones_bf")
            nc.vector.memset(ones_bf[:], 1.0)
            ones_f = tp.tile([1, 128], bf16, tag="ones_f")
            nc.vector.memset(ones_f[:], 1.0)
            eps_t = tp.tile([128, 1], f32, tag="eps")
            nc.vector.memset(eps_t[:], EPS)
            _core(nc, tc, s, nt, ntc, d_hid, d_wqaT, d_wkvaT, d_wqbT,
                  d_out, ident, cosT, sinT, masks, ones_bf, ones_f,
                  eps_t, d_qabs, d_oabsT, d_woT)

    _split_multi_waits(nc)
    return nc


def _core(nc, tc, s, nt, ntc, d_hid, d_wqaT, d_wkvaT, d_wqbT, d_out,
          ident, cosT, sinT, masks, ones_bf, ones_f, eps_t,
          d_qabs, d_oabsT, d_woT):
    MM = dict(skip_group_check=True)
    with tc.tile_pool(name="state", bufs=1) as st:
        kvT = [st.tile([128, 2, s], f8, tag=f"kvT{c}", name=f"kvT{c}")
               for c in range(2)]
        kpeT = st.tile([128, s], bf16, tag="kpeT")
        kv = [st.tile([128, 512], bf16, tag=f"kv{i}", name=f"kv{i}") for i in range(ntc)]
        qT_nope = [st.tile([128, s], bf16, tag=f"qTn{h}", name=f"qTn{h}")
                   for h in range(HPC)]
        q_peT = [st.tile([128, s], bf16, tag=f"qpeT{p}", name=f"qpeT{p}")
                 for p in range(HPC // 2)]

        # ---------------- phase AB: front-end projections ----------------
        scope_ab = nc.named_scope("ab")
        scope_ab.__enter__()
        with tc.tile_pool(name="ab", bufs=1) as ab, \
                tc.tile_pool(name="abp", bufs=1, space="PSUM") as abp:
            # DMA priority: hidden n=0 rows first so PE can start
            # transposing while the big weight tiles stream in.
            hbs = [ab.tile([128, HID], bf16, tag=f"hb{t4}", name=f"hb{t4}")
                   for t4 in range(4)]
            for t4 in range(4):
                nc.gpsimd.dma_start(out=hbs[t4][:],
                                    in_=d_hid[128 * t4:128 * (t4 + 1), :])
            wqaT_sb, wkvaT_sb = [], []
            for kk in range(8):
                t = ab.tile([128, 2, Q_LORA], f8, tag=f"wqa{kk}")
                nc.gpsimd.dma_start(out=t[:], in_=d_wqaT[kk])
                wqaT_sb.append(t)
            for k in range(16):
                t = ab.tile([128, 640], bf16, tag=f"wkva{k}")
                nc.gpsimd.dma_start(out=t[:], in_=d_wkvaT[128 * k:128 * (k + 1), :])
                wkvaT_sb.append(t)
            hidT = [ab.tile([128, 512], bf16, tag=f"hidT{k}",
                            name=f"hidT{k}") for k in range(16)]
            hidT8 = [ab.tile([128, 2, 512], f8, tag=f"hidT8_{kk}",
                             name=f"hidT8_{kk}") for kk in range(8)]
            # n=0: fine-grained per-(t4,k) transposes so t4=0 matmuls can
            # begin as soon as the first 128 hidden rows have landed
            for t4 in range(4):
                for k in range(16):
                    pt = abp.tile([128, 128], bf16, tag="pt", bufs=3,
                                  name="pt0")
                    nc.tensor.transpose(
                        pt[:], hbs[t4][:, 128 * k:128 * (k + 1)], ident[:])
                    tsl0 = slice(128 * t4, 128 * (t4 + 1))
                    if k % 2 == 0:
                        nc.scalar.copy(hidT[k][:, tsl0], pt[:])
                        nc.vector.tensor_copy(
                            hidT8[k // 2][:, k % 2, tsl0], pt[:])
                    else:
                        nc.vector.tensor_copy(hidT[k][:, tsl0], pt[:])
                        nc.scalar.copy(hidT8[k // 2][:, k % 2, tsl0], pt[:])

            def transpose_next_hid(n1):
                # prefetch + transpose hidden tile n1 into hidT; emitted
                # right after the wq_a matmuls of tile n1-1 so the PE fills
                # the rmsnorm latency bubble. WAR deps on hidT/hbs order
                # this safely behind the previous tile's matmul reads.
                for t4 in range(4):
                    r0 = 512 * n1 + 128 * t4
                    nc.gpsimd.dma_start(out=hbs[t4][:],
                                        in_=d_hid[r0:r0 + 128, :])
                for k in range(16):
                    pt = abp.tile([128, 512], bf16, tag="pt", bufs=3,
                                  name="pt")
                    for t4 in range(4):
                        nc.tensor.transpose(
                            pt[:, 128 * t4:128 * (t4 + 1)],
                            hbs[t4][:, 128 * k:128 * (k + 1)], ident[:])
                    if k % 2 == 0:
                        nc.scalar.copy(hidT[k][:], pt[:])
                        nc.vector.tensor_copy(hidT8[k // 2][:, k % 2, :],
                                              pt[:])
                    else:
                        nc.vector.tensor_copy(hidT[k][:], pt[:])
                        nc.scalar.copy(hidT8[k // 2][:, k % 2, :], pt[:])

            def emit_kv_rope(n_, krs_):
                ns_ = slice(512 * n_, 512 * (n_ + 1))
                # transpose kv -> kvT fp8 slices (kv normed in place)
                for c4 in range(4):
                    pt = abp.tile([128, 512], bf16, tag="pt", bufs=3,
                                  name="ptkv")
                    for t4 in range(4):
                        nc.tensor.transpose(
                            pt[:, 128 * t4:128 * (t4 + 1)],
                            kv[4 * n_ + t4][:, 128 * c4:128 * (c4 + 1)],
                            ident[:])
                    if c4 % 2 == 0:
                        nc.scalar.copy(kvT[c4 // 2][:, c4 % 2, ns_], pt[:])
                    else:
                        nc.vector.tensor_copy(kvT[c4 // 2][:, c4 % 2, ns_],
                                              pt[:])
                # k_pe / rot transposes, grouped the same way
                tpe = ab.tile([64, 512], bf16, tag="tpe", bufs=2)
                trot = ab.tile([64, 512], bf16, tag="trot", bufs=2)
                ppe = abp.tile([64, 512], bf16, tag="pt", bufs=3, name="ppe")
                for t4 in range(4):
                    nc.tensor.transpose(ppe[:, 128 * t4:128 * (t4 + 1)],
                                        krs_[t4][:, 0:64], ident[:])
                nc.scalar.copy(tpe[:], ppe[:])
                prot = abp.tile([64, 512], bf16, tag="pt", bufs=3, name="prot")
                for t4 in range(4):
                    nc.tensor.transpose(prot[:, 128 * t4:128 * (t4 + 1)],
                                        krs_[t4][:, 64:128], ident[:])
                nc.vector.tensor_copy(trot[:], prot[:])
                # rope for k_pe, write fp8 sub-tile 0, duplicate to the
                # upper 64 partitions for odd heads
                ta = ab.tile([64, 512], bf16, tag="ta", bufs=2)
                nc.vector.tensor_mul(kpeT[0:64, ns_], tpe[:], cosT[0:64, ns_])
                nc.vector.tensor_mul(ta[:], trot[:], sinT[0:64, ns_])
                nc.vector.tensor_add(kpeT[0:64, ns_], kpeT[0:64, ns_], ta[:])
                nc.sync.dma_start(out=kpeT[64:128, ns_],
                                  in_=kpeT[0:64, ns_])

            deferred = [None]
            for n in range(nt):
                ns = slice(512 * n, 512 * (n + 1))
                qanT = [ab.tile([128, 2, 512], f8, tag=f"qanT{kk}",
                                name=f"qanT{kk}") for kk in range(6)]
                qns, krs = [], []
                for t4 in range(4):
                    tsl = slice(128 * t4, 128 * (t4 + 1))
                    pqa = [abp.tile([128, 512], f32, tag=f"qa{f}",
                                    name=f"pqa{f}") for f in range(3)]
                    pck0 = abp.tile([128, 512], f32, tag="ck0")
                    pck1 = abp.tile([128, 128], f32, tag="ck1")
                    for kk in range(8):
                        st_, sp = (kk == 0), (kk == 7)
                        for f in range(3):
                            nc.tensor.matmul(
                                pqa[f][:], hidT8[kk][:, :, tsl],
                                wqaT_sb[kk][:, :, 512 * f:512 * (f + 1)],
                                start=st_, stop=sp, perf_mode=DR, **MM)
                        for k in (2 * kk, 2 * kk + 1):
                            nc.tensor.matmul(pck0[:], hidT[k][:, tsl],
                                             wkvaT_sb[k][:, 0:512],
                                             start=(k == 0), stop=(k == 15),
                                             **MM)
                            nc.tensor.matmul(pck1[:], hidT[k][:, tsl],
                                             wkvaT_sb[k][:, 512:640],
                                             start=(k == 0), stop=(k == 15),
                                             **MM)
                    # fast psum evac so next chunk's matmuls can start;
                    # rmsnorm runs on the bf16 copies, in place.
                    qn = ab.tile([128, Q_LORA], bf16, tag=f"qan{t4}",
                                 name=f"qn{t4}")
                    nc.scalar.copy(qn[:, 0:512], pqa[0][:])
                    nc.vector.tensor_copy(qn[:, 512:1024], pqa[1][:])
                    nc.scalar.copy(qn[:, 1024:1536], pqa[2][:])
                    nc.scalar.copy(kv[4 * n + t4][:], pck0[:])
                    kr = ab.tile([128, 128], bf16, tag=f"kpe{t4}", bufs=2,
                                 name=f"kr{t4}")
                    nc.vector.tensor_copy(kr[:], pck1[:])
                    qns.append(qn)
                    krs.append(kr)
                    # rmsnorm(q_a): stats over 1536, in-place scale
                    stats = ab.tile([128, 3, 6], f32, tag="stats")
                    for f in range(3):
                        nc.vector.bn_stats(out=stats[:, f, :],
                                           in_=qn[:, 512 * f:512 * (f + 1)])
                    mv = ab.tile([128, 2], f32, tag="mv")
                    nc.vector.bn_aggr(out=mv[:], in_=stats[:])
                    m2 = ab.tile([128, 1], f32, tag="m2")
                    nc.vector.tensor_mul(m2[:], mv[:, 0:1], mv[:, 0:1])
                    nc.vector.tensor_add(m2[:], m2[:], mv[:, 1:2])
                    nc.scalar.activation(out=m2[:], in_=m2[:], func=SQRT,
                                         bias=eps_t[:], scale=1.0)
                    rstd = ab.tile([128, 1], f32, tag="rstd")
                    nc.vector.reciprocal(out=rstd[:], in_=m2[:])
                    nc.vector.tensor_scalar_mul(qn[:], qn[:], rstd[:])
                    # rmsnorm(kv), in place on the state tile
                    stk = ab.tile([128, 6], f32, tag="stk")
                    nc.vector.bn_stats(out=stk[:], in_=kv[4 * n + t4][:])
                    mvk = ab.tile([128, 2], f32, tag="mvk")
                    nc.vector.bn_aggr(out=mvk[:], in_=stk[:])
                    m2k = ab.tile([128, 1], f32, tag="m2k")
                    nc.vector.tensor_mul(m2k[:], mvk[:, 0:1], mvk[:, 0:1])
                    nc.vector.tensor_add(m2k[:], m2k[:], mvk[:, 1:2])
                    nc.scalar.activation(out=m2k[:], in_=m2k[:], func=SQRT,
                                         bias=eps_t[:], scale=1.0)
                    rstdk = ab.tile([128, 1], f32, tag="rstdk")
                    nc.vector.reciprocal(out=rstdk[:], in_=m2k[:])
                    nc.vector.tensor_scalar_mul(kv[4 * n + t4][:],
                                                kv[4 * n + t4][:], rstdk[:])

                # prefetch + transpose next hidden tile (or the deferred
                # kv transposes of n-1): PE work that fills the wait for
                # the rmsnorm chain above
                if n + 1 < nt:
                    transpose_next_hid(n + 1)
                elif deferred[0] is not None:
                    emit_kv_rope(*deferred[0])
                    deferred[0] = None
                # transpose q_a_norm: group the 4 t4-chunks of each k into
                # one [128,512] psum -> 12 single-evac groups
                for k in range(12):
                    pt = abp.tile([128, 512], bf16, tag="pt", bufs=3,
                                  name="ptq")
                    for t4 in range(4):
                        nc.tensor.transpose(
                            pt[:, 128 * t4:128 * (t4 + 1)],
                            qns[t4][:, 128 * k:128 * (k + 1)], ident[:])
                    if k % 2 == 0:
                        nc.scalar.copy(qanT[k // 2][:, k % 2, :], pt[:])
                    else:
                        nc.vector.tensor_copy(qanT[k // 2][:, k % 2, :],
                                              pt[:])

                # wq_b projection: two head-pair passes, m-tiles per pass:
                # nopeA, nopeB, peA|peB, rotA|rotB
                for hp in range(HPC // 2):
                    pq = [abp.tile([128, 512], f32,
                                   tag=f"qa{m}" if m < 3 else "ck0",
                                   name=f"pq{m}") for m in range(4)]
                    for kk in range(6):
                        wq = ab.tile([128, 2, 512], f8, tag="wqb", bufs=3,
                                     name="wq")
                        nc.gpsimd.dma_start(
                            out=wq[:],
                            in_=d_wqbT[kk, :, :, 512 * hp:512 * (hp + 1)])
                        for m in range(4):
                            nc.tensor.matmul(pq[m][:],
                                             wq[:, :, 128 * m:128 * (m + 1)],
                                             qanT[kk][:], start=(kk == 0),
                                             stop=(kk == 5), **MM,
                                             perf_mode=DR)
                    nc.scalar.copy(qT_nope[2 * hp][:, ns], pq[0][:])
                    nc.vector.tensor_copy(qT_nope[2 * hp + 1][:, ns], pq[1][:])
                    qpe = ab.tile([128, 512], bf16, tag="qpe")
                    qrot = ab.tile([128, 512], bf16, tag="qrot")
                    nc.scalar.copy(qpe[:], pq[2][:])
                    nc.scalar.copy(qrot[:], pq[3][:])
                    ta2 = ab.tile([128, 512], bf16, tag="ta2")
                    nc.vector.tensor_mul(q_peT[hp][:, ns], qpe[:], cosT[:, ns])
                    nc.vector.tensor_mul(ta2[:], qrot[:], sinT[:, ns])
                    nc.vector.tensor_add(q_peT[hp][:, ns], q_peT[hp][:, ns],
                                         ta2[:])

                if n == 2:
                    deferred[0] = (2, krs)
                else:
                    emit_kv_rope(n, krs)


        scope_ab.__exit__(None, None, None)
        # duplicate roped k_pe into partitions 64..127
        nc.sync.dma_start(out=kpeT[64:128, :], in_=kpeT[0:64, :])

        # ---------------- phase C: attention + phase D: wo ----------------
        scope_at = nc.named_scope("at")
        scope_at.__enter__()
        with tc.tile_pool(name="at", bufs=1) as at, \
                tc.tile_pool(name="atp", bufs=1, space="PSUM") as atp:
            qabs_sb, oabsT_sb, woT_sb = [], [], []
            for h in range(HPC):
                q = at.tile([128, 512], bf16, tag=f"qabs{h}", name=f"qabs{h}")
                nc.gpsimd.dma_start(out=q[:], in_=d_qabs[h])
                qabs_sb.append(q)
                row = []
                for c4 in range(4):
                    t = at.tile([128, 128], bf16, tag=f"oabsT{h}_{c4}",
                                name=f"oabsT{h}_{c4}")
                    nc.gpsimd.dma_start(
                        out=t[:], in_=d_oabsT[h, 128 * c4:128 * (c4 + 1), :])
                    row.append(t)
                oabsT_sb.append(row)
                t = at.tile([128, HID], f32r, tag=f"woT{h}", name=f"woT{h}")
                nc.gpsimd.dma_start(
                    out=t[:], in_=d_woT[128 * h:128 * (h + 1), :])
                woT_sb.append(t)
            pending = [None]

            def finalize():
                if pending[0] is None:
                    return
                lsb_, xT_, fh_, y_ = pending[0]
                pending[0] = None
                pb = atp.tile([128, 512], f32, tag="s", bufs=3, name="pb")
                nc.tensor.matmul(pb[:], ones_f[:], lsb_[:],
                                 start=True, stop=True, **MM)
                linv = at.tile([128, 512], f32, tag="linv", bufs=2,
                               name="linv")
                nc.vector.tensor_copy(linv[:], pb[:])
                py = atp.tile([128, 512], f32, tag="s", bufs=3, name="py")
                for c4 in range(4):
                    nc.tensor.matmul(py[:], oabsT_sb[fh_][c4][:], xT_[c4][:],
                                     start=(c4 == 0), stop=(c4 == 3), **MM)
                nc.vector.tensor_mul(y_[:], py[:], linv[:])

            # j-outer: all heads process query tile j, then the wo
            # projection for those 512 tokens runs fused, so output DMA
            # overlaps attention of the next tile and y stays small.
            for j in range(nt):
                js = slice(512 * j, 512 * (j + 1))
                nblk = 4 * j + 4
                ys = []
                qaT8 = []
                for h in range(HPC):
                    tiles = [at.tile([128, 2, 512], f8, tag=f"qabsT{h}_{t}",
                                     bufs=2, name=f"qabsT{h}_{t}")
                             for t in range(2)]
                    for c4 in range(4):
                        p = atp.tile([128, 512], f32, tag="s", bufs=3)
                        nc.tensor.matmul(
                            p[:], qabs_sb[h][:, 128 * c4:128 * (c4 + 1)],
                            qT_nope[h][:, js], start=True, stop=True, **MM)
                        if c4 % 2 == 0:
                            nc.vector.tensor_copy(
                                tiles[c4 // 2][:, c4 % 2, :], p[:])
                        else:
                            nc.scalar.copy(
                                tiles[c4 // 2][:, c4 % 2, :], p[:])
                    qaT8.append(tiles)
                for h in range(HPC):
                    hs = 64 * (h % 2)
                    qpeT_h = q_peT[h // 2]
                    q_absT = qaT8[h]
                    po = [atp.tile([128, 512], f32, tag=f"o{c4}",
                                   name=f"po{c4}") for c4 in range(4)]
                    pl = atp.tile([1, 512], f32, tag="l")

                    def consume(i, ps):
                        # exp + (mask) + po/pl matmuls for score block i
                        pT = at.tile([128, 512], bf16, tag="pT", bufs=3)
                        nc.scalar.activation(out=pT[:], in_=ps[:], func=EXP,
                                             scale=SCALE / Q4)
                        if i >= 4 * j:
                            nc.vector.tensor_mul(pT[:], pT[:],
                                                 masks[i - 4 * j][:])
                        st_, sp = (i == 0), (i == nblk - 1)
                        for c4 in range(4):
                            nc.tensor.matmul(po[c4][:],
                                             kv[i][:, 128 * c4:128 * (c4 + 1)],
                                             pT[:], start=st_, stop=sp, **MM)
                        nc.tensor.matmul(pl[:], ones_bf[:], pT[:],
                                         start=st_, stop=sp, **MM)

                    prev = None
                    for i in range(nblk):
                        isl = slice(128 * i, 128 * (i + 1))
                        ps = atp.tile([128, 512], f32, tag="s", bufs=3)
                        for t in range(2):
                            nc.tensor.matmul(ps[:], kvT[t][:, :, isl],
                                             q_absT[t][:],
                                             start=(t == 0), stop=False,
                                             perf_mode=DR, **MM)
                        nc.tensor.matmul(
                            ps[:], kpeT[hs:hs + 64, isl],
                            qpeT_h[hs:hs + 64, js],
                            start=False, stop=True, **MM)
                        if prev is not None:
                            consume(*prev)
                        prev = (i, ps)
                        if i == 3:
                            finalize()
                    consume(*prev)
                    # quick psum evac; defer the dependent matmuls into the
                    # next head's score loop so PE never waits here
                    lsb = at.tile([1, 512], bf16, tag="lsb", bufs=2,
                                  name="lsb")
                    with nc.allow_low_precision(reason="1/l softmax scale"):
                        nc.vector.reciprocal(out=lsb[:], in_=pl[:])
                    xT = []
                    for c4 in range(4):
                        x = at.tile([128, 512], bf16, tag=f"xT{c4}", bufs=2,
                                    name=f"xT{c4}")
                        if c4 < 2:
                            nc.scalar.copy(x[:], po[c4][:])
                        else:
                            nc.vector.tensor_copy(x[:], po[c4][:])
                        xT.append(x)
                    y = at.tile([128, 512], f32r, tag=f"y{h}", bufs=2,
                                name=f"y{h}")
                    ys.append(y)
                    pending[0] = (lsb, xT, h, y)
                finalize()
                # fused wo for query tile j: out.T partial = woT.T @ (y/l)
                for m in range(16):
                    msl = slice(128 * m, 128 * (m + 1))
                    pw = atp.tile([128, 512], f32, tag=f"o{m % 4}",
                                  name="pw")
                    for kh in range(HPC):
                        nc.tensor.matmul(pw[:], woT_sb[kh][:, msl],
                                         ys[kh][:], start=(kh == 0),
                                         stop=(kh == HPC - 1), **MM)
                    ou = at.tile([128, 512], f32, tag="ou", bufs=3)
                    if m % 2 == 0:
                        nc.vector.tensor_copy(ou[:], pw[:])
                    else:
                        nc.scalar.copy(ou[:], pw[:])
                    nc.sync.dma_start(out=d_out[msl, js], in_=ou[:])
            scope_at.__exit__(None, None, None)


def _split_multi_waits(nc, limit=1):
    cnt = 0
    for f in nc.m.functions:
        for bb in f.blocks:
            newlist = []
            for inst in bb.instructions:
                si = inst.sync_info
                waits = list(si.on_wait) if si and si.on_wait else []
                if len(waits) > limit:
                    extra, keep = waits[:-limit], waits[-limit:]
                    for w in extra:
                        nop = mybir.InstNoOp(name=f"I-wsplit-{cnt}", ins=[],
                                             outs=[])
                        cnt += 1
                        nop.engine = inst.engine
                        nop.sync_info = mybir.SyncInfo(on_wait=[w], on_update=[])
                        newlist.append(nop)
                    inst.sync_info = mybir.SyncInfo(
                        on_wait=keep,
                        on_update=list(si.on_update) if si.on_update else [])
                newlist.append(inst)
            bb.instructions = newlist
    return cnt


# ----------------------------------------------------------------------
# host-side sharding / weight prep
# ----------------------------------------------------------------------
def _rope_tables(s):
    inv = 1.0 / (THETA ** (np.arange(0, ROPE_D, 2, dtype=np.float64) / ROPE_D))
    f = np.arange(s, dtype=np.float64)[:, None] * inv[None, :]  # [s, 32]
    emb = np.concatenate([f, f], axis=1)  # [s, 64]
    cosT = np.cos(emb).T.astype(np.float32)  # [64, s]
    sinT = np.sin(emb).T.astype(np.float32)
    return (np.concatenate([cosT, cosT], 0), np.concatenate([sinT, sinT], 0))


def _prep_in_maps(inputs, s=S):
    hid = np.asarray(inputs["hidden_states"], np.float32)
    wq_a = np.asarray(inputs["wq_a"], np.float32)
    q_ln = np.asarray(inputs["q_a_ln_w"], np.float32)
    wq_b = np.asarray(inputs["wq_b"], np.float32)
    wkv_a = np.asarray(inputs["wkv_a"], np.float32)
    kv_ln = np.asarray(inputs["kv_a_ln_w"], np.float32)
    wkv_b = np.asarray(inputs["wkv_b"], np.float32)
    wo = np.asarray(inputs["wo"], np.float32)

    perm = np.concatenate([np.arange(0, ROPE_D, 2), np.arange(1, ROPE_D, 2)])
    R = np.zeros((ROPE_D, ROPE_D), np.float32)
    R[np.arange(32), np.arange(32) + 32] = -1.0
    R[np.arange(32) + 32, np.arange(32)] = 1.0

    wqaT = np.ascontiguousarray(
        wq_a.T.reshape(8, 2, 128, Q_LORA).transpose(0, 2, 1, 3)
    ).astype(ml_dtypes.float8_e4m3)  # [8, 128, 2, Q_LORA]
    pe_kv = wkv_a[KV_LORA:][perm]  # [64, HID], permuted
    wkvaT = np.ascontiguousarray(
        np.concatenate([wkv_a[:KV_LORA], pe_kv, R @ pe_kv], 0).T
    ).astype(BF16NP)  # [HID, 640]

    cosT, sinT = _rope_tables(s)
    cosT = cosT.astype(BF16NP)
    sinT = sinT.astype(BF16NP)
    maskT = np.zeros((4, 128, 512), np.float32)
    for k in range(4):
        i = np.arange(128)[:, None] + 128 * k
        j = np.arange(512)[None, :]
        maskT[k] = (i <= j).astype(np.float32)
    maskT = maskT.astype(BF16NP)

    w = wkv_b.reshape(NH, NOPE + VH, KV_LORA)
    in_maps = []
    for core in range(NCORES):
        bid, hg = core // GROUP, core % GROUP
        heads = [HPC * hg + i for i in range(HPC)]
        rows = []
        for hp in range(HPC // 2):
            hA, hB = heads[2 * hp], heads[2 * hp + 1]
            nope_A = wq_b[hA * 192:hA * 192 + 128]
            nope_B = wq_b[hB * 192:hB * 192 + 128]
            pe_A = wq_b[hA * 192 + 128:hA * 192 + 192][perm]
            pe_B = wq_b[hB * 192 + 128:hB * 192 + 192][perm]
            rows.append(np.concatenate(
                [nope_A, nope_B, Q4 * pe_A, Q4 * pe_B,
                 Q4 * (R @ pe_A), Q4 * (R @ pe_B)], 0))
        wqb_eff = np.concatenate(rows, 0) * q_ln[None, :]  # [1024, QL]
        wqbT8 = np.ascontiguousarray(
            wqb_eff.T.reshape(6, 2, 128, 1024).transpose(0, 2, 1, 3)
        ).astype(ml_dtypes.float8_e4m3)  # [6, 128, 2, 1024]
        qabs = np.ascontiguousarray(
            w[heads, :NOPE, :] * kv_ln[None, None, :] * Q4).astype(BF16NP)
        oabs = w[heads, VH:, :] * kv_ln[None, None, :]  # [4, 128vh, 512c]
        oabsT = np.ascontiguousarray(oabs.transpose(0, 2, 1)).astype(BF16NP)
        woT = np.ascontiguousarray(
            wo[:, 512 * hg:512 * (hg + 1)].T)  # [512, HID]
        in_maps.append({
            "hidden": np.ascontiguousarray(hid[bid]).astype(BF16NP),
            "wqaT": wqaT,
            "wkvaT": wkvaT,
            "wqbT": wqbT8,
            "qabs": qabs,
            "oabsT": oabsT,
            "woT": woT,
            "cosT": cosT,
            "sinT": sinT,
            "maskT": maskT,
        })
    return in_maps


def kernel(**inputs):
    global LAST_EXEC_NS
    s = np.asarray(inputs["hidden_states"]).shape[1]
    if s not in _BUILD_CACHE:
        _BUILD_CACHE[s] = _build_program(s)
    nc = _BUILD_CACHE[s]
    in_maps = _prep_in_maps(inputs, s)
    res = run_bass_kernel_spmd(nc, in_maps, core_ids=list(range(NCORES)),
                               trace=False)
    LAST_EXEC_NS = res.exec_time_ns
    out = np.empty((B, s, HID), np.float32)
    for b in range(B):
        acc = res.results[GROUP * b]["out"].astype(np.float32)
        for i in range(1, GROUP):
            acc = acc + res.results[GROUP * b + i]["out"]
        out[b] = acc.T
    return out


# revision 16
# speedup vs baseline: 1.0971x; 1.0971x over previous
"""DeepseekV3 MLA attention prefill on 8 Trainium2 NeuronCores.

Sharding: batch x heads. Cores 0-3 run batch 0, cores 4-7 batch 1; within
a batch group each core owns 4 of the 16 heads. The replicated front-end
(q_a / kv_a projections) is computed once per core for its single batch
(vs twice in a pure head-parallel split), cutting per-core matmul work by
~27%. The final wo partial outputs are summed on the host per group.

Inside each core: transposed dataflow (contraction dims on SBUF
partitions), bf16 matmuls with f32 PSUM accumulation, f32r (full-rate
fp32) for the final wo projection, causal flash-style attention without
max-subtraction (scores ~N(0,0.3), exp is safe in f32). The score loop is
software-pipelined: score matmuls for block i+1 are emitted ahead of the
exp-dependent po matmuls of block i so the PE never idles on the scalar
engine. Rope is folded into host-side weight transforms.
"""
import os
import sys
import types

import numpy as np

# --- environment bootstrap (idempotent) --------------------------------
for _p in ("/opt/trn_rl_repo",):
    if os.path.isdir(_p) and _p not in sys.path:
        sys.path.insert(0, _p)
_B16 = ("/nix/store/wxap7svlj45h0lfm31d1axjjnzyl6qsy-b16-bazel-unstable-cc-"
        "2026-05-04-9a3fa1f3-rt-2026-05-04-ade39e0a/lib/python3.13/site-packages")
if os.path.isdir(_B16) and _B16 not in sys.path:
    sys.path.insert(0, _B16)

if "antenv.axon_hooks" not in sys.modules:
    try:
        import antenv

        _mod = types.ModuleType("antenv.axon_hooks")
        _hook = [None]
        _mod.set_axon_ntff_profile_hook = lambda h: _hook.__setitem__(0, h)
        _mod.get_axon_ntff_profile_hook = lambda: _hook[0]
        sys.modules["antenv.axon_hooks"] = _mod
        antenv.axon_hooks = _mod
        try:
            from trn_agent_boot.trn_boot import _ntff_profile_via_ctypes

            _mod.set_axon_ntff_profile_hook(
                _ntff_profile_via_ctypes("/opt/axon/libaxon_pjrt.so"))
        except Exception:
            pass
    except Exception:
        pass

import ml_dtypes

import concourse.bass as bass
import concourse.mybir as mybir
import concourse.tile as tile
from concourse.bass_utils import run_bass_kernel_spmd
from concourse.masks import make_identity

f32 = mybir.dt.float32
f32r = mybir.dt.float32r
bf16 = mybir.dt.bfloat16
f8 = mybir.dt.float8e4
EXP = mybir.ActivationFunctionType.Exp
DR = mybir.MatmulPerfMode.DoubleRow
SQRT = mybir.ActivationFunctionType.Sqrt
BF16NP = ml_dtypes.bfloat16

B, S, HID = 2, 2048, 2048
NH, NCORES = 16, 8
GROUP = 4             # cores per batch
HPC = NH // GROUP     # heads per core (4)
Q_LORA, KV_LORA = 1536, 512
NOPE, ROPE_D, VH = 128, 64, 128
EPS = 1e-6
THETA = 10000.0
SCALE = (NOPE + ROPE_D) ** -0.5
Q4 = 4.0               # q-side pre-scale so fp8 q_abs avoids denormals

LAST_EXEC_NS = None
_BUILD_CACHE = {}


# ----------------------------------------------------------------------
# device program (SPMD; one Bass program, per-core weights via in_maps)
# ----------------------------------------------------------------------
def _build_program(s=S):
    nt = s // 512          # 512-token tiles
    ntc = s // 128         # 128-token chunks

    nc = bass.Bass()
    d_hid = nc.declare_dram_parameter("hidden", [s, HID], bf16, isOutput=False)
    d_wqaT = nc.declare_dram_parameter("wqaT", [8, 128, 2, Q_LORA], f8,
                                       isOutput=False)
    d_wkvaT = nc.declare_dram_parameter("wkvaT", [HID, 640], bf16, isOutput=False)
    d_wqbT = nc.declare_dram_parameter("wqbT", [Q_LORA, 1024], bf16, isOutput=False)
    d_qabs = nc.declare_dram_parameter("qabs", [HPC, 128, 512], bf16, isOutput=False)
    d_oabsT = nc.declare_dram_parameter("oabsT", [HPC, 512, 128], bf16, isOutput=False)
    d_woT = nc.declare_dram_parameter("woT", [HPC * VH, HID], f32, isOutput=False)
    d_cosT = nc.declare_dram_parameter("cosT", [128, s], bf16, isOutput=False)
    d_sinT = nc.declare_dram_parameter("sinT", [128, s], bf16, isOutput=False)
    d_mask = nc.declare_dram_parameter("maskT", [4, 128, 512], bf16, isOutput=False)
    d_out = nc.declare_dram_parameter("out", [HID, s], f32, isOutput=True)

    with tile.TileContext(nc) as tc:
        with tc.tile_pool(name="tables", bufs=1) as tp:
            ident = tp.tile([128, 128], bf16, tag="ident")
            make_identity(nc, ident[:])
            cosT = tp.tile([128, s], bf16, tag="cosT")
            sinT = tp.tile([128, s], bf16, tag="sinT")
            nc.gpsimd.dma_start(out=cosT[:], in_=d_cosT[:])
            nc.gpsimd.dma_start(out=sinT[:], in_=d_sinT[:])
            masks = []
            for k in range(4):
                m = tp.tile([128, 512], bf16, tag=f"mask{k}")
                nc.gpsimd.dma_start(out=m[:], in_=d_mask[k])
                masks.append(m)
            ones_bf = tp.tile([128, 1], bf16, tag="ones_bf")
            nc.vector.memset(ones_bf[:], 1.0)
            ones_f = tp.tile([1, 128], bf16, tag="ones_f")
            nc.vector.memset(ones_f[:], 1.0)
            eps_t = tp.tile([128, 1], f32, tag="eps")
            nc.vector.memset(eps_t[:], EPS)
            _core(nc, tc, s, nt, ntc, d_hid, d_wqaT, d_wkvaT, d_wqbT,
                  d_out, ident, cosT, sinT, masks, ones_bf, ones_f,
                  eps_t, d_qabs, d_oabsT, d_woT)

    _split_multi_waits(nc)
    return nc


def _core(nc, tc, s, nt, ntc, d_hid, d_wqaT, d_wkvaT, d_wqbT, d_out,
          ident, cosT, sinT, masks, ones_bf, ones_f, eps_t,
          d_qabs, d_oabsT, d_woT):
    MM = dict(skip_group_check=True)
    with tc.tile_pool(name="state", bufs=1) as st:
        kvT = [st.tile([128, 2, s], f8, tag=f"kvT{c}", name=f"kvT{c}")
               for c in range(2)]
        kpeT = st.tile([128, s], bf16, tag="kpeT")
        kv = [st.tile([128, 512], bf16, tag=f"kv{i}", name=f"kv{i}") for i in range(ntc)]
        qT_nope = [st.tile([128, s], bf16, tag=f"qTn{h}", name=f"qTn{h}")
                   for h in range(HPC)]
        q_peT = [st.tile([128, s], bf16, tag=f"qpeT{p}", name=f"qpeT{p}")
                 for p in range(HPC // 2)]

        # ---------------- phase AB: front-end projections ----------------
        scope_ab = nc.named_scope("ab")
        scope_ab.__enter__()
        with tc.tile_pool(name="ab", bufs=1) as ab, \
                tc.tile_pool(name="abp", bufs=1, space="PSUM") as abp:
            # DMA priority: hidden n=0 rows first so PE can start
            # transposing while the big weight tiles stream in.
            hbs = [ab.tile([128, HID], bf16, tag=f"hb{t4}", name=f"hb{t4}")
                   for t4 in range(4)]
            for t4 in range(4):
                nc.gpsimd.dma_start(out=hbs[t4][:],
                                    in_=d_hid[128 * t4:128 * (t4 + 1), :])
            wqaT_sb, wkvaT_sb = [], []
            for kk in range(8):
                t = ab.tile([128, 2, Q_LORA], f8, tag=f"wqa{kk}")
                nc.gpsimd.dma_start(out=t[:], in_=d_wqaT[kk])
                wqaT_sb.append(t)
            for k in range(16):
                t = ab.tile([128, 640], bf16, tag=f"wkva{k}")
                nc.gpsimd.dma_start(out=t[:], in_=d_wkvaT[128 * k:128 * (k + 1), :])
                wkvaT_sb.append(t)
            hidT = [ab.tile([128, 512], bf16, tag=f"hidT{k}",
                            name=f"hidT{k}") for k in range(16)]
            hidT8 = [ab.tile([128, 2, 512], f8, tag=f"hidT8_{kk}",
                             name=f"hidT8_{kk}") for kk in range(8)]
            # n=0: fine-grained per-(t4,k) transposes so t4=0 matmuls can
            # begin as soon as the first 128 hidden rows have landed
            for t4 in range(4):
                for k in range(16):
                    pt = abp.tile([128, 128], bf16, tag="pt", bufs=3,
                                  name="pt0")
                    nc.tensor.transpose(
                        pt[:], hbs[t4][:, 128 * k:128 * (k + 1)], ident[:])
                    tsl0 = slice(128 * t4, 128 * (t4 + 1))
                    if k % 2 == 0:
                        nc.scalar.copy(hidT[k][:, tsl0], pt[:])
                        nc.vector.tensor_copy(
                            hidT8[k // 2][:, k % 2, tsl0], pt[:])
                    else:
                        nc.vector.tensor_copy(hidT[k][:, tsl0], pt[:])
                        nc.scalar.copy(hidT8[k // 2][:, k % 2, tsl0], pt[:])

            def transpose_next_hid(n1):
                # prefetch + transpose hidden tile n1 into hidT; emitted
                # right after the wq_a matmuls of tile n1-1 so the PE fills
                # the rmsnorm latency bubble. WAR deps on hidT/hbs order
                # this safely behind the previous tile's matmul reads.
                for t4 in range(4):
                    r0 = 512 * n1 + 128 * t4
                    nc.gpsimd.dma_start(out=hbs[t4][:],
                                        in_=d_hid[r0:r0 + 128, :])
                for k in range(16):
                    pt = abp.tile([128, 512], bf16, tag="pt", bufs=3,
                                  name="pt")
                    for t4 in range(4):
                        nc.tensor.transpose(
                            pt[:, 128 * t4:128 * (t4 + 1)],
                            hbs[t4][:, 128 * k:128 * (k + 1)], ident[:])
                    if k % 2 == 0:
                        nc.scalar.copy(hidT[k][:], pt[:])
                        nc.vector.tensor_copy(hidT8[k // 2][:, k % 2, :],
                                              pt[:])
                    else:
                        nc.vector.tensor_copy(hidT[k][:], pt[:])
                        nc.scalar.copy(hidT8[k // 2][:, k % 2, :], pt[:])

            def emit_kv_rope(n_, krs_):
                ns_ = slice(512 * n_, 512 * (n_ + 1))
                # transpose kv -> kvT fp8 slices (kv normed in place)
                for c4 in range(4):
                    pt = abp.tile([128, 512], bf16, tag="pt", bufs=3,
                                  name="ptkv")
                    for t4 in range(4):
                        nc.tensor.transpose(
                            pt[:, 128 * t4:128 * (t4 + 1)],
                            kv[4 * n_ + t4][:, 128 * c4:128 * (c4 + 1)],
                            ident[:])
                    if c4 % 2 == 0:
                        nc.scalar.copy(kvT[c4 // 2][:, c4 % 2, ns_], pt[:])
                    else:
                        nc.vector.tensor_copy(kvT[c4 // 2][:, c4 % 2, ns_],
                                              pt[:])
                # k_pe / rot transposes, grouped the same way
                tpe = ab.tile([64, 512], bf16, tag="tpe", bufs=2)
                trot = ab.tile([64, 512], bf16, tag="trot", bufs=2)
                ppe = abp.tile([64, 512], bf16, tag="pt", bufs=3, name="ppe")
                for t4 in range(4):
                    nc.tensor.transpose(ppe[:, 128 * t4:128 * (t4 + 1)],
                                        krs_[t4][:, 0:64], ident[:])
                nc.scalar.copy(tpe[:], ppe[:])
                prot = abp.tile([64, 512], bf16, tag="pt", bufs=3, name="prot")
                for t4 in range(4):
                    nc.tensor.transpose(prot[:, 128 * t4:128 * (t4 + 1)],
                                        krs_[t4][:, 64:128], ident[:])
                nc.vector.tensor_copy(trot[:], prot[:])
                # rope for k_pe, write fp8 sub-tile 0, duplicate to the
                # upper 64 partitions for odd heads
                ta = ab.tile([64, 512], bf16, tag="ta", bufs=2)
                nc.vector.tensor_mul(kpeT[0:64, ns_], tpe[:], cosT[0:64, ns_])
                nc.vector.tensor_mul(ta[:], trot[:], sinT[0:64, ns_])
                nc.vector.tensor_add(kpeT[0:64, ns_], kpeT[0:64, ns_], ta[:])
                nc.sync.dma_start(out=kpeT[64:128, ns_],
                                  in_=kpeT[0:64, ns_])

            deferred = [None]
            for n in range(nt):
                ns = slice(512 * n, 512 * (n + 1))
                qanT = [ab.tile([128, 512], bf16, tag=f"qanT{k}",
                                name=f"qanT{k}") for k in range(12)]
                qns, krs = [], []
                for t4 in range(4):
                    tsl = slice(128 * t4, 128 * (t4 + 1))
                    pqa = [abp.tile([128, 512], f32, tag=f"qa{f}",
                                    name=f"pqa{f}") for f in range(3)]
                    pck0 = abp.tile([128, 512], f32, tag="ck0")
                    pck1 = abp.tile([128, 128], f32, tag="ck1")
                    for kk in range(8):
                        st_, sp = (kk == 0), (kk == 7)
                        for f in range(3):
                            nc.tensor.matmul(
                                pqa[f][:], hidT8[kk][:, :, tsl],
                                wqaT_sb[kk][:, :, 512 * f:512 * (f + 1)],
                                start=st_, stop=sp, perf_mode=DR, **MM)
                        for k in (2 * kk, 2 * kk + 1):
                            nc.tensor.matmul(pck0[:], hidT[k][:, tsl],
                                             wkvaT_sb[k][:, 0:512],
                                             start=(k == 0), stop=(k == 15),
                                             **MM)
                            nc.tensor.matmul(pck1[:], hidT[k][:, tsl],
                                             wkvaT_sb[k][:, 512:640],
                                             start=(k == 0), stop=(k == 15),
                                             **MM)
                    # fast psum evac so next chunk's matmuls can start;
                    # rmsnorm runs on the bf16 copies, in place.
                    qn = ab.tile([128, Q_LORA], bf16, tag=f"qan{t4}",
                                 name=f"qn{t4}")
                    nc.scalar.copy(qn[:, 0:512], pqa[0][:])
                    nc.vector.tensor_copy(qn[:, 512:1024], pqa[1][:])
                    nc.scalar.copy(qn[:, 1024:1536], pqa[2][:])
                    nc.scalar.copy(kv[4 * n + t4][:], pck0[:])
                    kr = ab.tile([128, 128], bf16, tag=f"kpe{t4}", bufs=2,
                                 name=f"kr{t4}")
                    nc.vector.tensor_copy(kr[:], pck1[:])
                    qns.append(qn)
                    krs.append(kr)
                    # rmsnorm(q_a): stats over 1536, in-place scale
                    stats = ab.tile([128, 3, 6], f32, tag="stats")
                    for f in range(3):
                        nc.vector.bn_stats(out=stats[:, f, :],
                                           in_=qn[:, 512 * f:512 * (f + 1)])
                    mv = ab.tile([128, 2], f32, tag="mv")
                    nc.vector.bn_aggr(out=mv[:], in_=stats[:])
                    m2 = ab.tile([128, 1], f32, tag="m2")
                    nc.vector.tensor_mul(m2[:], mv[:, 0:1], mv[:, 0:1])
                    nc.vector.tensor_add(m2[:], m2[:], mv[:, 1:2])
                    nc.scalar.activation(out=m2[:], in_=m2[:], func=SQRT,
                                         bias=eps_t[:], scale=1.0)
                    rstd = ab.tile([128, 1], f32, tag="rstd")
                    nc.vector.reciprocal(out=rstd[:], in_=m2[:])
                    nc.vector.tensor_scalar_mul(qn[:], qn[:], rstd[:])
                    # rmsnorm(kv), in place on the state tile
                    stk = ab.tile([128, 6], f32, tag="stk")
                    nc.vector.bn_stats(out=stk[:], in_=kv[4 * n + t4][:])
                    mvk = ab.tile([128, 2], f32, tag="mvk")
                    nc.vector.bn_aggr(out=mvk[:], in_=stk[:])
                    m2k = ab.tile([128, 1], f32, tag="m2k")
                    nc.vector.tensor_mul(m2k[:], mvk[:, 0:1], mvk[:, 0:1])
                    nc.vector.tensor_add(m2k[:], m2k[:], mvk[:, 1:2])
                    nc.scalar.activation(out=m2k[:], in_=m2k[:], func=SQRT,
                                         bias=eps_t[:], scale=1.0)
                    rstdk = ab.tile([128, 1], f32, tag="rstdk")
                    nc.vector.reciprocal(out=rstdk[:], in_=m2k[:])
                    nc.vector.tensor_scalar_mul(kv[4 * n + t4][:],
                                                kv[4 * n + t4][:], rstdk[:])

                # prefetch + transpose next hidden tile (or the deferred
                # kv transposes of n-1): PE work that fills the wait for
                # the rmsnorm chain above
                if n + 1 < nt:
                    transpose_next_hid(n + 1)
                elif deferred[0] is not None:
                    emit_kv_rope(*deferred[0])
                    deferred[0] = None
                # transpose q_a_norm: group the 4 t4-chunks of each k into
                # one [128,512] psum -> 12 single-evac groups
                for k in range(12):
                    pt = abp.tile([128, 512], bf16, tag="pt", bufs=3,
                                  name="ptq")
                    for t4 in range(4):
                        nc.tensor.transpose(
                            pt[:, 128 * t4:128 * (t4 + 1)],
                            qns[t4][:, 128 * k:128 * (k + 1)], ident[:])
                    if k % 2 == 0:
                        nc.scalar.copy(qanT[k][:], pt[:])
                    else:
                        nc.vector.tensor_copy(qanT[k][:], pt[:])

                # wq_b projection: two head-pair passes, m-tiles per pass:
                # nopeA, nopeB, peA|peB, rotA|rotB
                for hp in range(HPC // 2):
                    pq = [abp.tile([128, 512], f32,
                                   tag=f"qa{m}" if m < 3 else "ck0",
                                   name=f"pq{m}") for m in range(4)]
                    for k in range(12):
                        wq = ab.tile([128, 512], bf16, tag="wqb", bufs=3,
                                     name="wq")
                        nc.gpsimd.dma_start(
                            out=wq[:],
                            in_=d_wqbT[128 * k:128 * (k + 1),
                                       512 * hp:512 * (hp + 1)])
                        for m in range(4):
                            nc.tensor.matmul(pq[m][:],
                                             wq[:, 128 * m:128 * (m + 1)],
                                             qanT[k][:], start=(k == 0),
                                             stop=(k == 11), **MM)
                    nc.scalar.copy(qT_nope[2 * hp][:, ns], pq[0][:])
                    nc.vector.tensor_copy(qT_nope[2 * hp + 1][:, ns], pq[1][:])
                    qpe = ab.tile([128, 512], bf16, tag="qpe")
                    qrot = ab.tile([128, 512], bf16, tag="qrot")
                    nc.scalar.copy(qpe[:], pq[2][:])
                    nc.scalar.copy(qrot[:], pq[3][:])
                    ta2 = ab.tile([128, 512], bf16, tag="ta2")
                    nc.vector.tensor_mul(q_peT[hp][:, ns], qpe[:], cosT[:, ns])
                    nc.vector.tensor_mul(ta2[:], qrot[:], sinT[:, ns])
                    nc.vector.tensor_add(q_peT[hp][:, ns], q_peT[hp][:, ns],
                                         ta2[:])

                if n == 2:
                    deferred[0] = (2, krs)
                else:
                    emit_kv_rope(n, krs)


        scope_ab.__exit__(None, None, None)
        # duplicate roped k_pe into partitions 64..127
        nc.sync.dma_start(out=kpeT[64:128, :], in_=kpeT[0:64, :])

        # ---------------- phase C: attention + phase D: wo ----------------
        scope_at = nc.named_scope("at")
        scope_at.__enter__()
        with tc.tile_pool(name="at", bufs=1) as at, \
                tc.tile_pool(name="atp", bufs=1, space="PSUM") as atp:
            qabs_sb, oabsT_sb, woT_sb = [], [], []
            for h in range(HPC):
                q = at.tile([128, 512], bf16, tag=f"qabs{h}", name=f"qabs{h}")
                nc.gpsimd.dma_start(out=q[:], in_=d_qabs[h])
                qabs_sb.append(q)
                row = []
                for c4 in range(4):
                    t = at.tile([128, 128], bf16, tag=f"oabsT{h}_{c4}",
                                name=f"oabsT{h}_{c4}")
                    nc.gpsimd.dma_start(
                        out=t[:], in_=d_oabsT[h, 128 * c4:128 * (c4 + 1), :])
                    row.append(t)
                oabsT_sb.append(row)
                t = at.tile([128, HID], f32r, tag=f"woT{h}", name=f"woT{h}")
                nc.gpsimd.dma_start(
                    out=t[:], in_=d_woT[128 * h:128 * (h + 1), :])
                woT_sb.append(t)
            pending = [None]

            def finalize():
                if pending[0] is None:
                    return
                lsb_, xT_, fh_, y_ = pending[0]
                pending[0] = None
                pb = atp.tile([128, 512], f32, tag="s", bufs=3, name="pb")
                nc.tensor.matmul(pb[:], ones_f[:], lsb_[:],
                                 start=True, stop=True, **MM)
                linv = at.tile([128, 512], f32, tag="linv", bufs=2,
                               name="linv")
                nc.vector.tensor_copy(linv[:], pb[:])
                py = atp.tile([128, 512], f32, tag="s", bufs=3, name="py")
                for c4 in range(4):
                    nc.tensor.matmul(py[:], oabsT_sb[fh_][c4][:], xT_[c4][:],
                                     start=(c4 == 0), stop=(c4 == 3), **MM)
                nc.vector.tensor_mul(y_[:], py[:], linv[:])

            # j-outer: all heads process query tile j, then the wo
            # projection for those 512 tokens runs fused, so output DMA
            # overlaps attention of the next tile and y stays small.
            for j in range(nt):
                js = slice(512 * j, 512 * (j + 1))
                nblk = 4 * j + 4
                ys = []
                qaT8 = []
                for h in range(HPC):
                    tiles = [at.tile([128, 2, 512], f8, tag=f"qabsT{h}_{t}",
                                     bufs=2, name=f"qabsT{h}_{t}")
                             for t in range(2)]
                    for c4 in range(4):
                        p = atp.tile([128, 512], f32, tag="s", bufs=3)
                        nc.tensor.matmul(
                            p[:], qabs_sb[h][:, 128 * c4:128 * (c4 + 1)],
                            qT_nope[h][:, js], start=True, stop=True, **MM)
                        if c4 % 2 == 0:
                            nc.vector.tensor_copy(
                                tiles[c4 // 2][:, c4 % 2, :], p[:])
                        else:
                            nc.scalar.copy(
                                tiles[c4 // 2][:, c4 % 2, :], p[:])
                    qaT8.append(tiles)
                for h in range(HPC):
                    hs = 64 * (h % 2)
                    qpeT_h = q_peT[h // 2]
                    q_absT = qaT8[h]
                    po = [atp.tile([128, 512], f32, tag=f"o{c4}",
                                   name=f"po{c4}") for c4 in range(4)]
                    pl = atp.tile([1, 512], f32, tag="l")

                    def consume(i, ps):
                        # exp + (mask) + po/pl matmuls for score block i
                        pT = at.tile([128, 512], bf16, tag="pT", bufs=3)
                        nc.scalar.activation(out=pT[:], in_=ps[:], func=EXP,
                                             scale=SCALE / Q4)
                        if i >= 4 * j:
                            nc.vector.tensor_mul(pT[:], pT[:],
                                                 masks[i - 4 * j][:])
                        st_, sp = (i == 0), (i == nblk - 1)
                        for c4 in range(4):
                            nc.tensor.matmul(po[c4][:],
                                             kv[i][:, 128 * c4:128 * (c4 + 1)],
                                             pT[:], start=st_, stop=sp, **MM)
                        nc.tensor.matmul(pl[:], ones_bf[:], pT[:],
                                         start=st_, stop=sp, **MM)

                    prev = None
                    for i in range(nblk):
                        isl = slice(128 * i, 128 * (i + 1))
                        ps = atp.tile([128, 512], f32, tag="s", bufs=3)
                        for t in range(2):
                            nc.tensor.matmul(ps[:], kvT[t][:, :, isl],
                                             q_absT[t][:],
                                             start=(t == 0), stop=False,
                                             perf_mode=DR, **MM)
                        nc.tensor.matmul(
                            ps[:], kpeT[hs:hs + 64, isl],
                            qpeT_h[hs:hs + 64, js],
                            start=False, stop=True, **MM)
                        if prev is not None:
                            consume(*prev)
                        prev = (i, ps)
                        if i == 3:
                            finalize()
                    consume(*prev)
                    # quick psum evac; defer the dependent matmuls into the
                    # next head's score loop so PE never waits here
                    lsb = at.tile([1, 512], bf16, tag="lsb", bufs=2,
                                  name="lsb")
                    with nc.allow_low_precision(reason="1/l softmax scale"):
                        nc.vector.reciprocal(out=lsb[:], in_=pl[:])
                    xT = []
                    for c4 in range(4):
                        x = at.tile([128, 512], bf16, tag=f"xT{c4}", bufs=2,
                                    name=f"xT{c4}")
                        if c4 < 2:
                            nc.scalar.copy(x[:], po[c4][:])
                        else:
                            nc.vector.tensor_copy(x[:], po[c4][:])
                        xT.append(x)
                    y = at.tile([128, 512], f32r, tag=f"y{h}", bufs=2,
                                name=f"y{h}")
                    ys.append(y)
                    pending[0] = (lsb, xT, h, y)
                finalize()
                # fused wo for query tile j: out.T partial = woT.T @ (y/l)
                for m in range(16):
                    msl = slice(128 * m, 128 * (m + 1))
                    pw = atp.tile([128, 512], f32, tag=f"o{m % 4}",
                                  name="pw")
                    for kh in range(HPC):
                        nc.tensor.matmul(pw[:], woT_sb[kh][:, msl],
                                         ys[kh][:], start=(kh == 0),
                                         stop=(kh == HPC - 1), **MM)
                    ou = at.tile([128, 512], f32, tag="ou", bufs=3)
                    if m % 2 == 0:
                        nc.vector.tensor_copy(ou[:], pw[:])
                    else:
                        nc.scalar.copy(ou[:], pw[:])
                    nc.sync.dma_start(out=d_out[msl, js], in_=ou[:])
            scope_at.__exit__(None, None, None)


def _split_multi_waits(nc, limit=1):
    cnt = 0
    for f in nc.m.functions:
        for bb in f.blocks:
            newlist = []
            for inst in bb.instructions:
                si = inst.sync_info
                waits = list(si.on_wait) if si and si.on_wait else []
                if len(waits) > limit:
                    extra, keep = waits[:-limit], waits[-limit:]
                    for w in extra:
                        nop = mybir.InstNoOp(name=f"I-wsplit-{cnt}", ins=[],
                                             outs=[])
                        cnt += 1
                        nop.engine = inst.engine
                        nop.sync_info = mybir.SyncInfo(on_wait=[w], on_update=[])
                        newlist.append(nop)
                    inst.sync_info = mybir.SyncInfo(
                        on_wait=keep,
                        on_update=list(si.on_update) if si.on_update else [])
                newlist.append(inst)
            bb.instructions = newlist
    return cnt


# ----------------------------------------------------------------------
# host-side sharding / weight prep
# ----------------------------------------------------------------------
def _rope_tables(s):
    inv = 1.0 / (THETA ** (np.arange(0, ROPE_D, 2, dtype=np.float64) / ROPE_D))
    f = np.arange(s, dtype=np.float64)[:, None] * inv[None, :]  # [s, 32]
    emb = np.concatenate([f, f], axis=1)  # [s, 64]
    cosT = np.cos(emb).T.astype(np.float32)  # [64, s]
    sinT = np.sin(emb).T.astype(np.float32)
    return (np.concatenate([cosT, cosT], 0), np.concatenate([sinT, sinT], 0))


def _prep_in_maps(inputs, s=S):
    hid = np.asarray(inputs["hidden_states"], np.float32)
    wq_a = np.asarray(inputs["wq_a"], np.float32)
    q_ln = np.asarray(inputs["q_a_ln_w"], np.float32)
    wq_b = np.asarray(inputs["wq_b"], np.float32)
    wkv_a = np.asarray(inputs["wkv_a"], np.float32)
    kv_ln = np.asarray(inputs["kv_a_ln_w"], np.float32)
    wkv_b = np.asarray(inputs["wkv_b"], np.float32)
    wo = np.asarray(inputs["wo"], np.float32)

    perm = np.concatenate([np.arange(0, ROPE_D, 2), np.arange(1, ROPE_D, 2)])
    R = np.zeros((ROPE_D, ROPE_D), np.float32)
    R[np.arange(32), np.arange(32) + 32] = -1.0
    R[np.arange(32) + 32, np.arange(32)] = 1.0

    wqaT = np.ascontiguousarray(
        wq_a.T.reshape(8, 2, 128, Q_LORA).transpose(0, 2, 1, 3)
    ).astype(ml_dtypes.float8_e4m3)  # [8, 128, 2, Q_LORA]
    pe_kv = wkv_a[KV_LORA:][perm]  # [64, HID], permuted
    wkvaT = np.ascontiguousarray(
        np.concatenate([wkv_a[:KV_LORA], pe_kv, R @ pe_kv], 0).T
    ).astype(BF16NP)  # [HID, 640]

    cosT, sinT = _rope_tables(s)
    cosT = cosT.astype(BF16NP)
    sinT = sinT.astype(BF16NP)
    maskT = np.zeros((4, 128, 512), np.float32)
    for k in range(4):
        i = np.arange(128)[:, None] + 128 * k
        j = np.arange(512)[None, :]
        maskT[k] = (i <= j).astype(np.float32)
    maskT = maskT.astype(BF16NP)

    w = wkv_b.reshape(NH, NOPE + VH, KV_LORA)
    in_maps = []
    for core in range(NCORES):
        bid, hg = core // GROUP, core % GROUP
        heads = [HPC * hg + i for i in range(HPC)]
        rows = []
        for hp in range(HPC // 2):
            hA, hB = heads[2 * hp], heads[2 * hp + 1]
            nope_A = wq_b[hA * 192:hA * 192 + 128]
            nope_B = wq_b[hB * 192:hB * 192 + 128]
            pe_A = wq_b[hA * 192 + 128:hA * 192 + 192][perm]
            pe_B = wq_b[hB * 192 + 128:hB * 192 + 192][perm]
            rows.append(np.concatenate(
                [nope_A, nope_B, Q4 * pe_A, Q4 * pe_B,
                 Q4 * (R @ pe_A), Q4 * (R @ pe_B)], 0))
        wqb_eff = np.concatenate(rows, 0) * q_ln[None, :]  # [1024, QL]
        qabs = np.ascontiguousarray(
            w[heads, :NOPE, :] * kv_ln[None, None, :] * Q4).astype(BF16NP)
        oabs = w[heads, VH:, :] * kv_ln[None, None, :]  # [4, 128vh, 512c]
        oabsT = np.ascontiguousarray(oabs.transpose(0, 2, 1)).astype(BF16NP)
        woT = np.ascontiguousarray(
            wo[:, 512 * hg:512 * (hg + 1)].T)  # [512, HID]
        in_maps.append({
            "hidden": np.ascontiguousarray(hid[bid]).astype(BF16NP),
            "wqaT": wqaT,
            "wkvaT": wkvaT,
            "wqbT": np.ascontiguousarray(wqb_eff.T).astype(BF16NP),
            "qabs": qabs,
            "oabsT": oabsT,
            "woT": woT,
            "cosT": cosT,
            "sinT": sinT,
            "maskT": maskT,
        })
    return in_maps


def kernel(**inputs):
    global LAST_EXEC_NS
    s = np.asarray(inputs["hidden_states"]).shape[1]
    if s not in _BUILD_CACHE:
        _BUILD_CACHE[s] = _build_program(s)
    nc = _BUILD_CACHE[s]
    in_maps = _prep_in_maps(inputs, s)
    res = run_bass_kernel_spmd(nc, in_maps, core_ids=list(range(NCORES)),
                               trace=False)
    LAST_EXEC_NS = res.exec_time_ns
    out = np.empty((B, s, HID), np.float32)
    for b in range(B):
        acc = res.results[GROUP * b]["out"].astype(np.float32)
        for i in range(1, GROUP):
            acc = acc + res.results[GROUP * b + i]["out"]
        out[b] = acc.T
    return out
